# revision 1
# baseline (speedup 1.0000x reference)
"""Bass/Tile TRN2 kernel for nn_BertEncoder_41592463294989.

4-layer BERT encoder, KERPLE attention bias, GLU MLP.
Sharding: data-parallel over batch (B=8 -> 8 cores, 1 sequence each).

Per-core layout: activations transposed [feature, token] so every matmul
contracts over the partition dim and LayerNorm reductions (over features)
are done with ones-vector matmuls on the PE.

Key tricks:
 - fp32r matmuls (full PE rate; ~1e-4 rounding).
 - softmax without max-subtraction (score magnitudes are bounded; the
   -10000 padding bias underflows exp to exactly 0, matching the reference).
 - scores computed transposed [key j, query i]; KERPLE bias tile computed
   elementwise from a constant log|i-j| table: exp/ln on ACT (one table set);
   the padding-mask bias enters as the per-partition bias of the softmax exp.
 - V weights are host-packed into per-head 65-column slots (64 features +
   a zero column whose bias is 1.0) so each PV matmul also produces the
   softmax denominator in psum row 64.
 - partition broadcasts (1/denominator, LN mu/rstd) via K=1 ones-matmuls.
 - GLU and the wo projection are fused: each 128-row x chunk feeds the wo
   accumulation immediately, so x never needs 6MB of SBUF.
"""
import contextlib

import numpy as np

import concourse.bass as bass
from concourse import bacc
import concourse.mybir as mybir
import concourse.tile as tile
from concourse.bass_utils import run_bass_kernel_spmd
from concourse.tile_rust import add_dep_helper

B, S, HID, NH, INTER, L = 8, 512, 768, 12, 3072, 4
DH = HID // NH          # 64
P = 128
NT = S // P             # 4 token tiles
KC = HID // P           # 6 hidden chunks
NIC = INTER // P        # 24 intermediate chunks
F32 = mybir.dt.float32
F32R = mybir.dt.float32r
AF = mybir.ActivationFunctionType
ALU = mybir.AluOpType

_BUILT = {}


def _prefer_combined_act_table(arch):
    """Steer the act-table-load pass to the natural_log_exp set for exp/ln.

    The placement pass greedily first-matches each activation function
    against the table list, so alternating exp/ln picks two different
    tables and reloads on every switch. Removing exp/ln from the
    single-function sets (in the cached dict, canonical indices unchanged)
    makes both resolve to the combined set -> no reloads. The emitted
    act_func_set_id still indexes the canonical act_info.json, and the
    combined table genuinely contains both functions.
    """
    from concourse.hw_specs import get_activation_tables
    tabs = get_activation_tables(arch)
    names = list(tabs)
    for nm in names:
        if nm == "natural_log_exp_and_others":
            continue
        tabs[nm].discard(AF.Exp)
        tabs[nm].discard(AF.Ln)


def _layernorm(nc, tc, z_t, out_t, g_t, b_t, ones_col, ones_row, z2p, smp):
    """LN over the feature (partition x chunk) axis of z_t [P, KC, S]."""
    EPS = 1e-12
    with tc.tile_pool(name="ln_ps", bufs=1, space="PSUM") as ln_ps, \
         tc.tile_pool(name="lnb_ps", bufs=1, space="PSUM") as lnb_ps:
        ps_sz = ln_ps.tile([1, S], F32, tag="sz")
        ps_sz2 = ln_ps.tile([1, S], F32, tag="sz2")
        for c in range(KC):
            nc.tensor.matmul(ps_sz[:], ones_col[:], z_t[:, c, :],
                             start=(c == 0), stop=(c == KC - 1))
        for c in range(KC):
            z2 = z2p.tile([P, S], F32R, tag="ztmp", name=f"zsq{c}")
            nc.vector.tensor_tensor(z2[:], z_t[:, c, :].bitcast(F32),
                                    z_t[:, c, :].bitcast(F32), ALU.mult)
            nc.tensor.matmul(ps_sz2[:], ones_col[:], z2[:],
                             start=(c == 0), stop=(c == KC - 1))
        mu = smp.tile([1, S], F32, tag="sm", name="mu")
        nc.vector.tensor_scalar(mu[:], ps_sz[:], 1.0 / HID, None, ALU.mult)
        m2 = smp.tile([1, S], F32, tag="sm", name="m2")
        nc.vector.tensor_scalar(m2[:], ps_sz2[:], 1.0 / HID, EPS, ALU.mult, ALU.add)
        var = smp.tile([1, S], F32, tag="sm", name="var")
        nc.vector.tensor_tensor(var[:], mu[:], mu[:], ALU.mult)
        nc.vector.tensor_tensor(var[:], m2[:], var[:], ALU.subtract)
        lnv = smp.tile([1, S], F32, tag="sm", name="lnv")
        nc.scalar.activation(lnv[:], var[:], AF.Ln, bias=0.0, scale=1.0)
        rstd = smp.tile([1, S], F32R, tag="sm", name="rstd")
        nc.scalar.activation(rstd[:], lnv[:], AF.Exp, bias=0.0, scale=-0.5)
        mu_r = smp.tile([1, S], F32R, tag="sm", name="mur")
        nc.vector.tensor_copy(mu_r[:], mu[:].bitcast(F32R))
        ps_mu = lnb_ps.tile([P, S], F32, tag="mub")
        nc.tensor.matmul(ps_mu[:], ones_row[:], mu_r[:], start=True, stop=True)
        ps_rs = lnb_ps.tile([P, S], F32, tag="rsb")
        nc.tensor.matmul(ps_rs[:], ones_row[:], rstd[:], start=True, stop=True)
        for c in range(KC):
            t1 = z2p.tile([P, S], F32, tag="ztmp", name=f"lnt{c}")
            nc.vector.tensor_tensor(t1[:], z_t[:, c, :].bitcast(F32), ps_mu[:],
                                    ALU.subtract)
            nc.vector.tensor_tensor(t1[:], t1[:], ps_rs[:], ALU.mult)
            nc.vector.tensor_scalar(out_t[:, c, :], t1[:], g_t[:, c:c + 1],
                                    b_t[:, c:c + 1], ALU.mult, ALU.add)


def _build(n_layers: int):
    nc = bacc.Bacc("TRN2", target_bir_lowering=False)
    try:
        _prefer_combined_act_table(nc.m.arch)
    except Exception:
        pass

    def inp(name, shape):
        return nc.declare_dram_parameter(name, list(shape), F32, isOutput=False)

    hT_d = inp("hT", [HID, S])
    mb_d = inp("mb", [P, NT])
    maskb_d = inp("maskb", [P, S])
    LR_d = inp("LR", [P, NT, S])
    ones_row_d = inp("ones_row", [1, P])
    ones_col_d = inp("ones_col", [P, 1])
    rp_d = inp("rp", [L, 36])
    wqk_d = inp("wqk", [L, 2 * KC, P, KC, P])
    bqk_d = inp("bqk", [L, P, 2 * KC])
    wva_d = inp("wva", [L, 2, P, KC, NH * 65 // 2])
    bva_d = inp("bva", [L, 1, NH * 65])
    woa_d = inp("woa", [L, KC, P, KC, P])
    boa_d = inp("boa", [L, P, KC])
    ln1g_d = inp("ln1g", [L, P, KC])
    ln1b_d = inp("ln1b", [L, P, KC])
    glu_d = inp("glu", [L, NIC, P, KC, 256])
    wot_d = inp("wot", [L, INTER, HID])
    bwo_d = inp("bwo", [L, P, KC])
    ln2g_d = inp("ln2g", [L, P, KC])
    ln2b_d = inp("ln2b", [L, P, KC])
    out_d = nc.declare_dram_parameter("out", [HID, S], F32, isOutput=True)

    with tile.TileContext(nc) as tc:
        lp = nc.allow_low_precision(reason="fp32r rounding for matmul operands")
        lp.__enter__()
        stack = contextlib.ExitStack()
        const = stack.enter_context(tc.tile_pool(name="const", bufs=1))
        hpool = stack.enter_context(tc.tile_pool(name="hpool", bufs=2))
        qkp = stack.enter_context(tc.tile_pool(name="qkp", bufs=1))
        vap = stack.enter_context(tc.tile_pool(name="vap", bufs=1))
        p4p = stack.enter_context(tc.tile_pool(name="p4p", bufs=2))
        kbp = stack.enter_context(tc.tile_pool(name="kbp", bufs=5))
        up = stack.enter_context(tc.tile_pool(name="up", bufs=2))
        atp = stack.enter_context(tc.tile_pool(name="atp", bufs=1))
        smp = stack.enter_context(tc.tile_pool(name="smp", bufs=3))
        zp = stack.enter_context(tc.tile_pool(name="zp", bufs=1))
        z2p = stack.enter_context(tc.tile_pool(name="z2p", bufs=2))
        aop = stack.enter_context(tc.tile_pool(name="aop", bufs=1))
        xcp = stack.enter_context(tc.tile_pool(name="xcp", bufs=2))
        xgp = stack.enter_context(tc.tile_pool(name="xgp", bufs=2))
        wst = stack.enter_context(tc.tile_pool(name="wst", bufs=2))   # [128,128] stream
        wvp = stack.enter_context(tc.tile_pool(name="wvp", bufs=1))   # wva halves
        wgp = stack.enter_context(tc.tile_pool(name="wgp", bufs=2))   # glu [128,256]
        wop = stack.enter_context(tc.tile_pool(name="wop", bufs=2))   # wot [128,768]
        bp = stack.enter_context(tc.tile_pool(name="bp", bufs=2))
        bvp = stack.enter_context(tc.tile_pool(name="bvp", bufs=1))
        prp = stack.enter_context(tc.tile_pool(name="prp", bufs=2))

        # ---- constants ----
        LR_t = const.tile([P, NT, S], F32)
        nc.sync.dma_start(LR_t[:], LR_d[:])
        mb_t = const.tile([P, NT], F32)
        nc.sync.dma_start(mb_t[:], mb_d[:])
        maskb_t = const.tile([P, S], F32)
        nc.sync.dma_start(maskb_t[:], maskb_d[:])
        ones_row = const.tile([1, P], F32R)
        nc.sync.dma_start(ones_row[:], ones_row_d[:].bitcast(F32R))
        ones_col = const.tile([P, 1], F32R)
        nc.sync.dma_start(ones_col[:], ones_col_d[:].bitcast(F32R))

        # layer 0 hidden state
        h_t = hpool.tile([P, KC, S], F32R, tag="h")
        nc.sync.dma_start(h_t[:], hT_d[:].rearrange("(c p) t -> p c t", p=P).bitcast(F32R))

        last_gelu = [None]
        first_kerple = [None]
        prev_exp = [None]
        for l in range(n_layers):
            # ---------- per-layer r-params: broadcast + clip ----------
            with tc.tile_pool(name="pp", bufs=1, space="PSUM") as ppp:
                rp_r = prp.tile([1, 36], F32R, tag="rp_r")
                nc.sync.dma_start(rp_r[:], rp_d[l, None, :].bitcast(F32R))
                ps_rp = ppp.tile([P, 36], F32)
                nc.tensor.matmul(ps_rp[:], ones_row[:], rp_r[:], start=True, stop=True)
                par = prp.tile([P, 36], F32, tag="par")
                nc.vector.tensor_scalar(par[:], ps_rp[:], 1e-7, None, ALU.max)
                nc.vector.tensor_scalar(par[:, 0:12], par[:, 0:12], -1.0, None, ALU.mult)
            # par: [:,0:12] = -c1, [:,12:24] = c2, [:,24:36] = c3

            with tc.tile_pool(name="qkv_ps", bufs=1, space="PSUM") as qkv_ps, \
                 tc.tile_pool(name="sc_ps", bufs=4, space="PSUM") as sc_ps, \
                 tc.tile_pool(name="pv_ps", bufs=2, space="PSUM") as pv_ps, \
                 tc.tile_pool(name="bc_ps", bufs=1, space="PSUM") as bc_ps:
                # ---------- QK ----------
                bqk_t = bp.tile([P, 2 * KC], F32, tag="bqk")
                nc.sync.dma_start(bqk_t[:], bqk_d[l])
                qk_t = qkp.tile([P, 2 * KC, S], F32R, tag="qk")
                for ot in range(2 * KC):
                    ps = qkv_ps.tile([P, S], F32, tag="qkvps")
                    w = wst.tile([P, KC, P], F32R, tag="w", name=f"wqk{ot}")
                    nc.sync.dma_start(w[:], wqk_d[l, ot].bitcast(F32R))
                    for kc in range(KC):
                        nc.tensor.matmul(ps[:], w[:, kc, :], h_t[:, kc, :],
                                         start=(kc == 0), stop=(kc == KC - 1))
                    nc.vector.tensor_scalar(qk_t[:, ot, :], ps[:],
                                            bqk_t[:, ot:ot + 1], None, ALU.add)

                # ---------- V (token-major, head-slotted + ones col) ----------
                bva_t = bvp.tile([1, NH * 65], F32R, tag="bva")
                nc.sync.dma_start(bva_t[:], bva_d[l].bitcast(F32R))
                va_t = vap.tile([P, NT, NH * 65], F32R, tag="va")
                HALF = NH * 65 // 2  # 390
                for half in range(2):
                    sl = slice(half * HALF, (half + 1) * HALF)
                    wv = wvp.tile([P, KC, HALF], F32R, tag="wv", name=f"wv{half}")
                    nc.sync.dma_start(wv[:], wva_d[l, half].bitcast(F32R))
                    for jt in range(NT):
                        ps = qkv_ps.tile([P, HALF], F32, tag="qkvps", name=f"vps{half}_{jt}")
                        for kc in range(KC):
                            nc.tensor.matmul(ps[:], h_t[:, kc, jt * P:(jt + 1) * P],
                                             wv[:, kc, :], start=(kc == 0), stop=False)
                        nc.tensor.matmul(ps[:], ones_row[:], bva_t[:, sl],
                                         start=False, stop=True)
                        nc.vector.tensor_copy(va_t[:, jt, sl], ps[:])

                # ---------- attention per head ----------
                at_t = atp.tile([P, KC, S], F32R, tag="attnT")
                for h in range(NH):
                    kslot = (HID + DH * h) // P
                    koff = (DH * h) % P
                    qslot = (DH * h) // P
                    qoff = (DH * h) % P
                    p4 = p4p.tile([P, NT, S], F32R, tag="p4")
                    ps_pv = pv_ps.tile([65, S], F32, tag="pv")
                    ps_ss = []
                    for jt in range(NT):
                        ps_s = sc_ps.tile([P, S], F32, tag="sc", name=f"sc{jt}")
                        ps_ss.append(ps_s)
                        nc.tensor.matmul(
                            ps_s[:],
                            qk_t[koff:koff + DH, kslot, jt * P:(jt + 1) * P],
                            qk_t[qoff:qoff + DH, qslot, :],
                            start=True, stop=True)
                    t1s = []
                    last_t1 = None
                    for jt in range(NT):
                        t1 = kbp.tile([P, S], F32, tag="kb1", name=f"kb1_{jt}")
                        t1s.append(t1)
                        _i = nc.scalar.activation(t1[:], LR_t[:, jt, :], AF.Exp,
                                                  bias=0.0, scale=par[:, 24 + h:25 + h])
                        if jt == 0:
                            if first_kerple[0] is None:
                                first_kerple[0] = _i
                                if last_gelu[0] is not None:
                                    add_dep_helper(_i.ins, last_gelu[0].ins, False,
                                                   "act table grouping")
                            if prev_exp[0] is not None:
                                add_dep_helper(_i.ins, prev_exp[0].ins, False,
                                               "act block order")
                        last_t1 = _i
                    last_ln = None
                    for jt in range(NT):
                        t2 = up.tile([P, S], F32, tag="kb2", name=f"kb2_{jt}")
                        _i = nc.scalar.activation(t2[:], t1s[jt][:], AF.Ln,
                                                  bias=1.0, scale=par[:, 12 + h:13 + h])
                        if jt == 0:
                            add_dep_helper(_i.ins, last_t1.ins, False,
                                           "act block order")
                        last_ln = _i
                        nc.vector.tensor_scalar(t2[:], t2[:], par[:, h:h + 1],
                                                None, ALU.mult)
                        nc.vector.tensor_tensor(ps_ss[jt][:], ps_ss[jt][:], t2[:],
                                                ALU.add)
                    for jt in range(NT):
                        _i = nc.scalar.activation(p4[:, jt, :], ps_ss[jt][:], AF.Exp,
                                                  bias=mb_t[:, jt:jt + 1], scale=1.0)
                        if jt == 0:
                            add_dep_helper(_i.ins, last_ln.ins, False,
                                           "act block order")
                        prev_exp[0] = _i
                    for jt in range(NT):
                        nc.tensor.matmul(ps_pv[:], va_t[:, jt, 65 * h:65 * h + 65],
                                         p4[:, jt, :], start=(jt == 0), stop=(jt == NT - 1))
                    rec = smp.tile([1, S], F32R, tag="sm", name="rec")
                    nc.vector.reciprocal(rec[:], ps_pv[64:65, :])
                    ps_bc = bc_ps.tile([64, S], F32)
                    nc.tensor.matmul(ps_bc[:], ones_row[:, 0:64], rec[:],
                                     start=True, stop=True)
                    rb_sb = up.tile([64, S], F32, tag="rb", name="rb_sb")
                    nc.vector.tensor_copy(rb_sb[:], ps_bc[:])
                    nc.vector.tensor_tensor(
                        at_t[64 * (h % 2):64 * (h % 2) + 64, h // 2, :],
                        ps_pv[0:64, :], rb_sb[:], ALU.mult)

                # ---------- attention out projection + residual ----------
                boa_t = bp.tile([P, KC], F32, tag="boa")
                nc.sync.dma_start(boa_t[:], boa_d[l])
                ln1g_t = bp.tile([P, KC], F32, tag="ln1g")
                nc.sync.dma_start(ln1g_t[:], ln1g_d[l])
                ln1b_t = bp.tile([P, KC], F32, tag="ln1b")
                nc.sync.dma_start(ln1b_t[:], ln1b_d[l])
                z_t = zp.tile([P, KC, S], F32R, tag="z")
                for ot in range(KC):
                    ps = sc_ps.tile([P, S], F32, tag="sc", name=f"prj{ot}")
                    w = wst.tile([P, KC, P], F32R, tag="w", name=f"woa{ot}")
                    nc.sync.dma_start(w[:], woa_d[l, ot].bitcast(F32R))
                    for kc in range(KC):
                        nc.tensor.matmul(ps[:], w[:, kc, :], at_t[:, kc, :],
                                         start=(kc == 0), stop=(kc == KC - 1))
                    zt = z2p.tile([P, S], F32, tag="z1", name=f"z1_{ot}")
                    nc.vector.tensor_scalar(zt[:], ps[:], boa_t[:, ot:ot + 1],
                                            None, ALU.add)
                    nc.vector.tensor_tensor(z_t[:, ot, :], zt[:],
                                            h_t[:, ot, :].bitcast(F32), ALU.add)

            # ---------- LN1 ----------
            ao_t = aop.tile([P, KC, S], F32R, tag="ao")
            _layernorm(nc, tc, z_t, ao_t, ln1g_t, ln1b_t, ones_col, ones_row,
                       z2p, smp)

            # ---------- GLU + wo (fused) ----------
            with tc.tile_pool(name="glu_ps", bufs=1, space="PSUM") as glu_ps, \
                 tc.tile_pool(name="wo_ps", bufs=6, space="PSUM") as wo_ps:
                bwo_t = bp.tile([P, KC], F32, tag="bwo")
                nc.sync.dma_start(bwo_t[:], bwo_d[l])
                ln2g_t = bp.tile([P, KC], F32, tag="ln2g")
                nc.sync.dma_start(ln2g_t[:], ln2g_d[l])
                ln2b_t = bp.tile([P, KC], F32, tag="ln2b")
                nc.sync.dma_start(ln2b_t[:], ln2b_d[l])

                wo_acc = [wo_ps.tile([P, S], F32, tag="woacc", name=f"woacc{i}")
                          for i in range(KC)]
                for gt in range(NIC):
                    ps_g = glu_ps.tile([P, S], F32, tag="gps")
                    ps_u = glu_ps.tile([P, S], F32, tag="ups")
                    gw = wgp.tile([P, KC, 256], F32R, tag="gw", name=f"gw{gt}")
                    nc.sync.dma_start(gw[:], glu_d[l, gt].bitcast(F32R))
                    for kc in range(KC):
                        nc.tensor.matmul(ps_g[:], gw[:, kc, 0:128], ao_t[:, kc, :],
                                         start=(kc == 0), stop=(kc == KC - 1))
                    for kc in range(KC):
                        nc.tensor.matmul(ps_u[:], gw[:, kc, 128:256], ao_t[:, kc, :],
                                         start=(kc == 0), stop=(kc == KC - 1))
                    xg = xgp.tile([P, S], F32, tag="xg")
                    last_gelu[0] = nc.scalar.activation(xg[:], ps_g[:], AF.Gelu)
                    first_kerple[0] = None
                    xc = xcp.tile([P, S], F32R, tag="xc")
                    nc.vector.tensor_tensor(xc[:], xg[:], ps_u[:], ALU.mult)
                    wot_t = wop.tile([P, HID], F32R, tag="wot")
                    nc.sync.dma_start(wot_t[:], wot_d[l, gt * P:(gt + 1) * P, :].bitcast(F32R))
                    for ot in range(KC):
                        nc.tensor.matmul(wo_acc[ot][:], wot_t[:, ot * P:(ot + 1) * P],
                                         xc[:], start=(gt == 0), stop=(gt == NIC - 1))

                z2_t = zp.tile([P, KC, S], F32R, tag="z", name="z_mlp")
                for ot in range(KC):
                    zt = z2p.tile([P, S], F32, tag="z1", name=f"z2_{ot}")
                    nc.vector.tensor_scalar(zt[:], wo_acc[ot][:], bwo_t[:, ot:ot + 1],
                                            None, ALU.add)
                    nc.vector.tensor_tensor(z2_t[:, ot, :], zt[:],
                                            ao_t[:, ot, :].bitcast(F32), ALU.add)

            # ---------- LN2 -> next h ----------
            h_t = hpool.tile([P, KC, S], F32R, tag="h", name=f"h{l + 1}")
            _layernorm(nc, tc, z2_t, h_t, ln2g_t, ln2b_t, ones_col, ones_row,
                       z2p, smp)

        # ---------- final mask + store ----------
        out_sb = zp.tile([P, KC, S], F32, tag="z", name="out_sb")
        for c in range(KC):
            nc.vector.tensor_tensor(out_sb[:, c, :], h_t[:, c, :].bitcast(F32),
                                    maskb_t[:], ALU.mult)
        nc.sync.dma_start(out_d[:].rearrange("(c p) t -> p c t", p=P), out_sb[:])

        stack.close()
        lp.__exit__(None, None, None)

    nc.finalize()
    return nc


def _prep_inputs(hidden_states, attention_mask, Wqkv_w, Wqkv_b, attn_out_w,
                 attn_out_b, ln1_g, ln1_b, glu_w, wo_w, wo_b, ln2_g, ln2_b,
                 r1, r2, r3):
    """Host-side sharding + weight layout transforms (shared across cores)."""
    f32 = np.float32
    shared = {}
    idx_i = np.arange(S)
    LR = np.empty((P, NT, S), f32)
    for jt in range(NT):
        jglob = jt * P + np.arange(P)
        d = np.abs(idx_i[None, :] - jglob[:, None]).astype(np.float64)
        with np.errstate(divide="ignore"):
            lg = np.log(d)
        lg[d == 0] = -1e30
        LR[:, jt, :] = lg.astype(f32)
    shared["LR"] = LR
    shared["ones_row"] = np.ones((1, P), f32)
    shared["ones_col"] = np.ones((P, 1), f32)
    shared["rp"] = np.concatenate(
        [r1.reshape(L, NH), r2.reshape(L, NH), r3.reshape(L, NH)], axis=1
    ).astype(f32)

    wq = Wqkv_w[:, :HID, :] / 8.0           # fold 1/sqrt(DH)
    wk = Wqkv_w[:, HID:2 * HID, :]
    bq = Wqkv_b[:, :HID] / 8.0
    bk = Wqkv_b[:, HID:2 * HID]
    wqk = np.concatenate([wq, wk], axis=1)  # [L, 1536, HID]
    wqkT = np.transpose(wqk, (0, 2, 1))  # [L, HID, 1536]
    shared["wqk"] = np.ascontiguousarray(
        wqkT.reshape(L, KC, P, 2 * KC, P).transpose(0, 3, 2, 1, 4)).astype(f32)
    bqk = np.concatenate([bq, bk], axis=1)  # [L, 1536]
    shared["bqk"] = np.ascontiguousarray(
        bqk.reshape(L, 2 * KC, P).transpose(0, 2, 1)).astype(f32)

    wv = Wqkv_w[:, 2 * HID:, :]             # [L, 768v, 768]
    bv = Wqkv_b[:, 2 * HID:]
    wva = np.zeros((L, HID, NH * 65), f32)
    bva = np.zeros((L, 1, NH * 65), f32)
    for h in range(NH):
        wva[:, :, 65 * h:65 * h + 64] = np.transpose(
            wv[:, DH * h:DH * (h + 1), :], (0, 2, 1))
        bva[:, 0, 65 * h:65 * h + 64] = bv[:, DH * h:DH * (h + 1)]
        bva[:, 0, 65 * h + 64] = 1.0
    shared["wva"] = np.ascontiguousarray(
        wva.reshape(L, KC, P, 2, NH * 65 // 2).transpose(0, 3, 2, 1, 4)).astype(f32)
    shared["bva"] = bva

    woaT = np.transpose(attn_out_w, (0, 2, 1))  # [L, HID, HID]
    shared["woa"] = np.ascontiguousarray(
        woaT.reshape(L, KC, P, KC, P).transpose(0, 3, 2, 1, 4)).astype(f32)

    def pcol(v):  # [L, 768] -> [L, P, KC]
        return np.ascontiguousarray(v.reshape(L, KC, P).transpose(0, 2, 1)).astype(f32)

    shared["boa"] = pcol(attn_out_b)
    shared["ln1g"] = pcol(ln1_g)
    shared["ln1b"] = pcol(ln1_b)
    glu = np.empty((L, HID, NIC, 256), f32)
    gw = np.transpose(glu_w, (0, 2, 1))     # [L, HID, 6144]
    for gt in range(NIC):
        glu[:, :, gt, 0:128] = gw[:, :, gt * P:(gt + 1) * P]
        glu[:, :, gt, 128:256] = gw[:, :, INTER + gt * P:INTER + (gt + 1) * P]
    shared["glu"] = np.ascontiguousarray(
        glu.reshape(L, KC, P, NIC, 256).transpose(0, 3, 2, 1, 4)).astype(f32)
    shared["wot"] = np.ascontiguousarray(np.transpose(wo_w, (0, 2, 1))).astype(f32)
    shared["bwo"] = pcol(wo_b)
    shared["ln2g"] = pcol(ln2_g)
    shared["ln2b"] = pcol(ln2_b)

    in_maps = []
    for b in range(B):
        m = dict(shared)
        m["hT"] = np.ascontiguousarray(hidden_states[b].T).astype(f32)
        mask = attention_mask[b].astype(f32)          # [S]
        mbias = (1.0 - mask) * -10000.0
        m["mb"] = np.ascontiguousarray(mbias.reshape(NT, P).T).astype(f32)
        m["maskb"] = np.broadcast_to(mask[None, :], (P, S)).copy()
        in_maps.append(m)
    return in_maps


def kernel(**inputs) -> np.ndarray:
    n_layers = int(inputs.pop("_n_layers", L))
    if n_layers not in _BUILT:
        _BUILT[n_layers] = _build(n_layers)
    nc = _BUILT[n_layers]
    in_maps = _prep_inputs(**inputs)
    res = run_bass_kernel_spmd(nc, in_maps, list(range(B))).results
    out = np.empty((B, S, HID), np.float32)
    for b in range(B):
        out[b] = res[b]["out"].T
    return out



# revision 13
# speedup vs baseline: 1.2870x; 1.2870x over previous
"""Bass/Tile TRN2 kernel for nn_BertEncoder_41592463294989.

4-layer BERT encoder, KERPLE attention bias, GLU MLP.
Sharding: data-parallel over batch (B=8 -> 8 cores, 1 sequence each).

Per-core layout: activations transposed [feature, token] so every matmul
contracts over the partition dim and LayerNorm reductions (over features)
are done with ones-vector matmuls on the PE.

v2 design:
 - KERPLE bias is Toeplitz (depends only on |i-j|): exp(bias) is
   precomputed on the HOST per (layer, head) and shipped as a bf16 DRAM
   table; softmax becomes p = exp(s + padmask)*ekb. This removes all
   per-element exp/ln/pow work for the bias on the device (was 2/3 of
   ACT + half of attention DVE time).
 - All matmul operands (weights and activations) are bf16: full PE rate,
   half the weight-DMA bytes, 2x DVE rate on bf16 elementwise ops. The
   residual stream (z, h, ao) stays fp32; bf16 shadow copies feed matmuls.
 - All linear-layer biases are folded into the PE via rank-1 matmuls
   (bias row stationary, ones vector moving) instead of DVE/ACT adds.
 - V weights host-packed into per-head 65-column slots (64 features + a
   ones column) so each PV matmul also produces the softmax denominator.
 - partition broadcasts (1/denominator, LN mu/rstd) via K=1 ones-matmuls.
 - GLU and the wo projection are fused per 128-row chunk.
 - Weights packed into few DRAM tensors (dispatch cost scales with arg
   count in the PJRT path).
"""
import contextlib

import numpy as np
import ml_dtypes

import concourse.bass as bass
from concourse import bacc
import concourse.mybir as mybir
import concourse.tile as tile
from concourse.bass_utils import run_bass_kernel_spmd
from concourse.tile_rust import add_dep_helper

B, S, HID, NH, INTER, L = 8, 512, 768, 12, 3072, 4
DH = HID // NH          # 64
P = 128
NT = S // P             # 4 token tiles
KC = HID // P           # 6 hidden chunks
NIC = INTER // P        # 24 intermediate chunks
F32 = mybir.dt.float32
F32R = mybir.dt.float32r
BF16 = mybir.dt.bfloat16
NPBF16 = ml_dtypes.bfloat16
AF = mybir.ActivationFunctionType
ALU = mybir.AluOpType
HALF = NH * 65 // 2     # 390

_BUILT = {}


def _prefer_combined_act_table(arch):
    """Steer the act-table-load pass to the natural_log_exp set for exp/ln.

    The placement pass greedily first-matches each activation function
    against the table list, so alternating exp/ln picks two different
    tables and reloads on every switch. Removing exp/ln from the
    single-function sets (in the cached dict, canonical indices unchanged)
    makes both resolve to the combined set -> no reloads. The emitted
    act_func_set_id still indexes the canonical act_info.json, and the
    combined table genuinely contains both functions.
    """
    from concourse.hw_specs import get_activation_tables
    tabs = get_activation_tables(arch)
    for nm in list(tabs):
        if nm == "natural_log_exp_and_others":
            continue
        tabs[nm].discard(AF.Exp)
        tabs[nm].discard(AF.Ln)


def _layernorm(nc, tc, z_t, out_t, out16_t, g_t, b_t, ones_col, ones_row,
               z2p, smp):
    """LN over the feature (partition x chunk) axis of z_t [P, KC, S] (F32R).

    Writes F32R out_t and (if not None) bf16 out16_t (matmul operand shadow).
    """
    EPS = 1e-12
    with tc.tile_pool(name="ln_ps", bufs=1, space="PSUM") as ln_ps, \
         tc.tile_pool(name="lnb_ps", bufs=1, space="PSUM") as lnb_ps:
        ps_sz = ln_ps.tile([1, S], F32, tag="sz")
        ps_sz2 = ln_ps.tile([1, S], F32, tag="sz2")
        for c in range(KC):
            nc.tensor.matmul(ps_sz[:], ones_col[:], z_t[:, c, :],
                             start=(c == 0), stop=(c == KC - 1))
        for c in range(KC):
            z2 = z2p.tile([P, S], F32R, tag="ztmp", name=f"zsq{c}")
            nc.vector.tensor_tensor(z2[:], z_t[:, c, :].bitcast(F32),
                                    z_t[:, c, :].bitcast(F32), ALU.mult)
            nc.tensor.matmul(ps_sz2[:], ones_col[:], z2[:],
                             start=(c == 0), stop=(c == KC - 1))
        mu = smp.tile([1, S], F32, tag="sm", name="mu")
        nc.vector.tensor_scalar(mu[:], ps_sz[:], 1.0 / HID, None, ALU.mult)
        m2 = smp.tile([1, S], F32, tag="sm", name="m2")
        nc.vector.tensor_scalar(m2[:], ps_sz2[:], 1.0 / HID, EPS, ALU.mult, ALU.add)
        var = smp.tile([1, S], F32, tag="sm", name="var")
        nc.vector.tensor_tensor(var[:], mu[:], mu[:], ALU.mult)
        nc.vector.tensor_tensor(var[:], m2[:], var[:], ALU.subtract)
        lnv = smp.tile([1, S], F32, tag="sm", name="lnv")
        nc.scalar.activation(lnv[:], var[:], AF.Ln, bias=0.0, scale=1.0)
        rstd = smp.tile([1, S], F32R, tag="sm", name="rstd")
        nc.scalar.activation(rstd[:], lnv[:], AF.Exp, bias=0.0, scale=-0.5)
        mu_r = smp.tile([1, S], F32R, tag="sm", name="mur")
        nc.vector.tensor_copy(mu_r[:], mu[:].bitcast(F32R))
        ps_mu = lnb_ps.tile([P, S], F32, tag="mub")
        nc.tensor.matmul(ps_mu[:], ones_row[:], mu_r[:], start=True, stop=True)
        ps_rs = lnb_ps.tile([P, S], F32, tag="rsb")
        nc.tensor.matmul(ps_rs[:], ones_row[:], rstd[:], start=True, stop=True)
        for c in range(KC):
            t1 = z2p.tile([P, S], F32, tag="ztmp", name=f"lnt{c}")
            nc.vector.tensor_tensor(t1[:], z_t[:, c, :].bitcast(F32), ps_mu[:],
                                    ALU.subtract)
            nc.vector.tensor_tensor(t1[:], t1[:], ps_rs[:], ALU.mult)
            nc.vector.tensor_scalar(out_t[:, c, :], t1[:], g_t[:, c:c + 1],
                                    b_t[:, c:c + 1], ALU.mult, ALU.add)
            if out16_t is not None:
                nc.vector.tensor_copy(out16_t[:, c, :],
                                      out_t[:, c, :].bitcast(F32))


def _build(n_layers: int):
    nc = bacc.Bacc("TRN2", target_bir_lowering=False)
    try:
        _prefer_combined_act_table(nc.m.arch)
    except Exception:
        pass

    def inp(name, shape, dt=F32):
        return nc.declare_dram_parameter(name, list(shape), dt, isOutput=False)

    # fp32 consts: hT | mb | maskb | ones_row | ones_col | ln params
    hT_d = inp("hT", [HID, S])
    c32_d = inp("c32", [P, NT + S + 2 + 4 * L * KC])
    ones_row_d = inp("ones_row", [1, P])
    ones_col_d = inp("ones_col", [P, 1])
    # bf16: big weight blob, per-layer layout (offsets in elements):
    #   wqk [2KC, P, KC, P] | wva [2, P, KC, HALF] | woa [KC, P, KC, P]
    #   glu [NIC, P, KC, 256] | wot [INTER, HID]
    #   bqk [2KC*P] | bva [NH*65] | boa [KC*P] | bwo [KC*P]
    W_QKV = 2 * KC * P * KC * P
    W_V = 2 * P * KC * HALF
    W_OA = KC * P * KC * P
    W_GLU = NIC * P * KC * 256
    W_OT = INTER * HID
    W_B = 2 * KC * P + NH * 65 + KC * P + KC * P
    WLAY = W_QKV + W_V + W_OA + W_GLU + W_OT + W_B
    w16_d = inp("w16", [L * WLAY], BF16)
    h16_d = inp("h16", [HID, S], BF16)
    ones_s_d = inp("ones_s", [1, S], BF16)
    ekb_d = inp("ekb", [L, NH, P, NT * S], BF16)
    out_d = nc.declare_dram_parameter("out", [HID, S], F32, isOutput=True)

    def wslice(l, off, sz, shape):
        base = l * WLAY + off
        pat = " ".join(f"d{i}" for i in range(len(shape)))
        return w16_d[base:base + sz].rearrange(
            f"({pat}) -> {pat}", **{f"d{i}": shape[i] for i in range(len(shape))})

    O_QKV = 0
    O_V = O_QKV + W_QKV
    O_OA = O_V + W_V
    O_GLU = O_OA + W_OA
    O_OT = O_GLU + W_GLU
    O_BQK = O_OT + W_OT
    O_BVA = O_BQK + 2 * KC * P
    O_BOA = O_BVA + NH * 65
    O_BWO = O_BOA + KC * P

    with tile.TileContext(nc) as tc:
        lp = nc.allow_low_precision(reason="bf16 matmul operands; loose tol")
        lp.__enter__()
        stack = contextlib.ExitStack()
        const = stack.enter_context(tc.tile_pool(name="const", bufs=1))
        hpool = stack.enter_context(tc.tile_pool(name="hpool", bufs=2))
        h16p = stack.enter_context(tc.tile_pool(name="h16p", bufs=2))
        qkp = stack.enter_context(tc.tile_pool(name="qkp", bufs=1))
        vap = stack.enter_context(tc.tile_pool(name="vap", bufs=1))
        p4p = stack.enter_context(tc.tile_pool(name="p4p", bufs=2))
        ekbp = stack.enter_context(tc.tile_pool(name="ekbp", bufs=3))
        up = stack.enter_context(tc.tile_pool(name="up", bufs=2))
        atp = stack.enter_context(tc.tile_pool(name="atp", bufs=1))
        smp = stack.enter_context(tc.tile_pool(name="smp", bufs=3))
        zp = stack.enter_context(tc.tile_pool(name="zp", bufs=1))
        z2p = stack.enter_context(tc.tile_pool(name="z2p", bufs=2))
        aop = stack.enter_context(tc.tile_pool(name="aop", bufs=1))
        ao16p = stack.enter_context(tc.tile_pool(name="ao16p", bufs=1))
        xcp = stack.enter_context(tc.tile_pool(name="xcp", bufs=2))
        xgp = stack.enter_context(tc.tile_pool(name="xgp", bufs=2))
        wst = stack.enter_context(tc.tile_pool(name="wst", bufs=3))   # [128,KC,128] stream
        wvp = stack.enter_context(tc.tile_pool(name="wvp", bufs=2))   # wva halves
        wgp = stack.enter_context(tc.tile_pool(name="wgp", bufs=3))   # glu [128,KC,256]
        wop = stack.enter_context(tc.tile_pool(name="wop", bufs=3))   # wot [128,768]
        bp = stack.enter_context(tc.tile_pool(name="bp", bufs=2))
        bvp = stack.enter_context(tc.tile_pool(name="bvp", bufs=1))

        # ---- constants ----
        c32_t = const.tile([P, NT + S + 2 + 4 * L * KC], F32)
        nc.sync.dma_start(c32_t[:], c32_d[:])
        mb_t = c32_t[:, 0:NT]
        maskb_t = c32_t[:, NT:NT + S]
        lnp_t = c32_t[:, NT + S + 2:]   # [P, 4*L*KC]: l1g|l1b|l2g|l2b per layer
        ones_row = const.tile([1, P], F32R)
        nc.sync.dma_start(ones_row[:], ones_row_d[:].bitcast(F32R))
        ones_col = const.tile([P, 1], F32R)
        nc.sync.dma_start(ones_col[:], ones_col_d[:].bitcast(F32R))
        ones_s = const.tile([1, S], BF16)
        nc.sync.dma_start(ones_s[:], ones_s_d[:])

        # layer 0 hidden state (fp32 residual + bf16 matmul shadow)
        h_t = hpool.tile([P, KC, S], F32R, tag="h")
        nc.sync.dma_start(h_t[:], hT_d[:].rearrange("(c p) t -> p c t",
                                                    p=P).bitcast(F32R))
        h16_t = h16p.tile([P, KC, S], BF16, tag="h16")
        nc.sync.dma_start(h16_t[:], h16_d[:].rearrange("(c p) t -> p c t", p=P))

        last_gelu = [None]
        prev_exp = [None]
        for l in range(n_layers):
            ln1g_t = lnp_t[:, (4 * l) * KC:(4 * l + 1) * KC]
            ln1b_t = lnp_t[:, (4 * l + 1) * KC:(4 * l + 2) * KC]
            ln2g_t = lnp_t[:, (4 * l + 2) * KC:(4 * l + 3) * KC]
            ln2b_t = lnp_t[:, (4 * l + 3) * KC:(4 * l + 4) * KC]

            with tc.tile_pool(name="qkv_ps", bufs=2, space="PSUM") as qkv_ps, \
                 tc.tile_pool(name="sc_ps", bufs=4, space="PSUM") as sc_ps, \
                 tc.tile_pool(name="pv_ps", bufs=2, space="PSUM") as pv_ps:
                # ---------- QK ----------
                bqk_t = bp.tile([1, 2 * KC, P], BF16, tag="bqk")
                nc.sync.dma_start(bqk_t[:],
                                  wslice(l, O_BQK, 2 * KC * P, (1, 2 * KC, P)))
                qk_t = qkp.tile([P, 2 * KC, S], BF16, tag="qk")
                for ot in range(2 * KC):
                    ps = qkv_ps.tile([P, S], F32, tag="qkvps")
                    w = wst.tile([P, KC, P], BF16, tag="w", name=f"wqk{ot}")
                    nc.sync.dma_start(
                        w[:], wslice(l, O_QKV + ot * P * KC * P, P * KC * P,
                                     (P, KC, P)))
                    for kc in range(KC):
                        nc.tensor.matmul(ps[:], w[:, kc, :], h16_t[:, kc, :],
                                         start=(kc == 0), stop=False)
                    nc.tensor.matmul(ps[:], bqk_t[:, ot, :], ones_s[:],
                                     start=False, stop=True)
                    nc.vector.tensor_copy(qk_t[:, ot, :], ps[:])

                # ---------- V (token-major, head-slotted + ones col) ----------
                bva_t = bvp.tile([1, NH * 65], BF16, tag="bva")
                nc.sync.dma_start(bva_t[:], wslice(l, O_BVA, NH * 65, (1, NH * 65)))
                va_t = vap.tile([P, NT, NH * 65], BF16, tag="va")
                for half in range(2):
                    sl = slice(half * HALF, (half + 1) * HALF)
                    wv = wvp.tile([P, KC, HALF], BF16, tag="wv", name=f"wv{half}")
                    nc.sync.dma_start(
                        wv[:], wslice(l, O_V + half * P * KC * HALF, P * KC * HALF,
                                      (P, KC, HALF)))
                    for jt in range(NT):
                        ps = qkv_ps.tile([P, HALF], F32, tag="qkvps",
                                         name=f"vps{half}_{jt}")
                        for kc in range(KC):
                            nc.tensor.matmul(ps[:], h16_t[:, kc, jt * P:(jt + 1) * P],
                                             wv[:, kc, :], start=(kc == 0), stop=False)
                        nc.tensor.matmul(ps[:], ones_s[:, 0:P],
                                         bva_t[:, sl], start=False, stop=True)
                        nc.vector.tensor_copy(va_t[:, jt, sl], ps[:])

                # ---------- attention per head ----------
                at16 = atp.tile([P, KC, S], BF16, tag="attnT")
                for h in range(NH):
                    kslot = (HID + DH * h) // P
                    koff = (DH * h) % P
                    qslot = (DH * h) // P
                    qoff = (DH * h) % P
                    ekb_t = ekbp.tile([P, NT * S], BF16, tag="ekb", name=f"ekb{h}")
                    nc.sync.dma_start(ekb_t[:], ekb_d[l, h])
                    p4 = p4p.tile([P, NT, S], BF16, tag="p4")
                    ps_pv = pv_ps.tile([65, S], F32, tag="pv")
                    for jt in range(NT):
                        ps_s = sc_ps.tile([P, S], F32, tag="sc", name=f"sc{jt}")
                        nc.tensor.matmul(
                            ps_s[:],
                            qk_t[koff:koff + DH, kslot, jt * P:(jt + 1) * P],
                            qk_t[qoff:qoff + DH, qslot, :],
                            start=True, stop=True)
                        _i = nc.scalar.activation(p4[:, jt, :], ps_s[:], AF.Exp,
                                                  bias=mb_t[:, jt:jt + 1], scale=1.0)
                        if h == 0 and jt == 0 and last_gelu[0] is not None:
                            add_dep_helper(_i.ins, last_gelu[0].ins, False,
                                           "act table grouping")
                        prev_exp[0] = _i
                        nc.vector.tensor_tensor(
                            p4[:, jt, :], p4[:, jt, :],
                            ekb_t[:, jt * S:(jt + 1) * S], ALU.mult)
                        nc.tensor.matmul(ps_pv[:], va_t[:, jt, 65 * h:65 * h + 65],
                                         p4[:, jt, :], start=(jt == 0),
                                         stop=(jt == NT - 1))
                    rec = smp.tile([1, S], F32R, tag="sm", name="rec")
                    nc.vector.reciprocal(rec[:], ps_pv[64:65, :])
                    ps_bc = sc_ps.tile([64, S], F32, tag="sc", name="bc")
                    nc.tensor.matmul(ps_bc[:], ones_row[:, 0:64], rec[:],
                                     start=True, stop=True)
                    rb_sb = up.tile([64, S], F32, tag="rb", name="rb_sb")
                    nc.vector.tensor_copy(rb_sb[:], ps_bc[:])
                    nc.vector.tensor_tensor(
                        at16[64 * (h % 2):64 * (h % 2) + 64, h // 2, :],
                        ps_pv[0:64, :], rb_sb[:], ALU.mult)

                # ---------- attention out projection + residual ----------
                boa_t = bp.tile([1, KC, P], BF16, tag="boa")
                nc.sync.dma_start(boa_t[:], wslice(l, O_BOA, KC * P, (1, KC, P)))
                z_t = zp.tile([P, KC, S], F32R, tag="z")
                for ot in range(KC):
                    ps = sc_ps.tile([P, S], F32, tag="sc", name=f"prj{ot}")
                    w = wst.tile([P, KC, P], BF16, tag="w", name=f"woa{ot}")
                    nc.sync.dma_start(
                        w[:], wslice(l, O_OA + ot * P * KC * P, P * KC * P,
                                     (P, KC, P)))
                    for kc in range(KC):
                        nc.tensor.matmul(ps[:], w[:, kc, :], at16[:, kc, :],
                                         start=(kc == 0), stop=False)
                    nc.tensor.matmul(ps[:], boa_t[:, ot, :], ones_s[:],
                                     start=False, stop=True)
                    nc.vector.tensor_tensor(z_t[:, ot, :], ps[:],
                                            h_t[:, ot, :].bitcast(F32), ALU.add)

            # ---------- LN1 ----------
            ao_t = aop.tile([P, KC, S], F32R, tag="ao")
            ao16 = ao16p.tile([P, KC, S], BF16, tag="ao16")
            _layernorm(nc, tc, z_t, ao_t, ao16, ln1g_t, ln1b_t, ones_col,
                       ones_row, z2p, smp)

            # ---------- GLU + wo (fused) ----------
            with tc.tile_pool(name="glu_ps", bufs=1, space="PSUM") as glu_ps, \
                 tc.tile_pool(name="wo_ps", bufs=6, space="PSUM") as wo_ps:
                bwo_t = bp.tile([1, KC, P], BF16, tag="bwo")
                nc.sync.dma_start(bwo_t[:], wslice(l, O_BWO, KC * P, (1, KC, P)))

                wo_acc = [wo_ps.tile([P, S], F32, tag="woacc", name=f"woacc{i}")
                          for i in range(KC)]
                for gt in range(NIC):
                    ps_g = glu_ps.tile([P, S], F32, tag="gps")
                    ps_u = glu_ps.tile([P, S], F32, tag="ups")
                    gw = wgp.tile([P, KC, 256], BF16, tag="gw", name=f"gw{gt}")
                    nc.sync.dma_start(
                        gw[:], wslice(l, O_GLU + gt * P * KC * 256, P * KC * 256,
                                      (P, KC, 256)))
                    for kc in range(KC):
                        nc.tensor.matmul(ps_g[:], gw[:, kc, 0:128], ao16[:, kc, :],
                                         start=(kc == 0), stop=(kc == KC - 1))
                    for kc in range(KC):
                        nc.tensor.matmul(ps_u[:], gw[:, kc, 128:256], ao16[:, kc, :],
                                         start=(kc == 0), stop=(kc == KC - 1))
                    xg = xgp.tile([P, S], BF16, tag="xg")
                    _i = nc.scalar.activation(xg[:], ps_g[:], AF.Gelu)
                    if gt == 0 and prev_exp[0] is not None:
                        add_dep_helper(_i.ins, prev_exp[0].ins, False,
                                       "act table grouping")
                    last_gelu[0] = _i
                    xc = xcp.tile([P, S], BF16, tag="xc")
                    nc.vector.tensor_tensor(xc[:], xg[:], ps_u[:], ALU.mult)
                    wot_t = wop.tile([P, HID], BF16, tag="wot")
                    nc.sync.dma_start(
                        wot_t[:], wslice(l, O_OT + gt * P * HID, P * HID, (P, HID)))
                    for ot in range(KC):
                        nc.tensor.matmul(wo_acc[ot][:], wot_t[:, ot * P:(ot + 1) * P],
                                         xc[:], start=(gt == 0), stop=False)
                z2_t = zp.tile([P, KC, S], F32R, tag="z", name="z_mlp")
                for ot in range(KC):
                    nc.tensor.matmul(wo_acc[ot][:], bwo_t[:, ot, :], ones_s[:],
                                     start=False, stop=True)
                    nc.vector.tensor_tensor(z2_t[:, ot, :], wo_acc[ot][:],
                                            ao_t[:, ot, :].bitcast(F32), ALU.add)

            # ---------- LN2 -> next h ----------
            h_t = hpool.tile([P, KC, S], F32R, tag="h", name=f"h{l + 1}")
            if l + 1 < n_layers:
                h16_t = h16p.tile([P, KC, S], BF16, tag="h16",
                                  name=f"h16_{l + 1}")
            else:
                h16_t = None
            _layernorm(nc, tc, z2_t, h_t, h16_t, ln2g_t, ln2b_t, ones_col,
                       ones_row, z2p, smp)

        # ---------- final mask + store ----------
        out_sb = zp.tile([P, KC, S], F32, tag="z", name="out_sb")
        for c in range(KC):
            nc.vector.tensor_tensor(out_sb[:, c, :], h_t[:, c, :].bitcast(F32),
                                    maskb_t[:], ALU.mult)
        nc.sync.dma_start(out_d[:].rearrange("(c p) t -> p c t", p=P), out_sb[:])

        stack.close()
        lp.__exit__(None, None, None)

    nc.finalize()
    return nc


def _prep_inputs(hidden_states, attention_mask, Wqkv_w, Wqkv_b, attn_out_w,
                 attn_out_b, ln1_g, ln1_b, glu_w, wo_w, wo_b, ln2_g, ln2_b,
                 r1, r2, r3):
    """Host-side sharding + weight layout transforms (shared across cores)."""
    f32 = np.float32
    shared = {}
    shared["ones_row"] = np.ones((1, P), f32)
    shared["ones_col"] = np.ones((P, 1), f32)
    shared["ones_s"] = np.ones((1, S), NPBF16)

    # ekb: exp(kerple bias) per (layer, head), Toeplitz [S, S] -> [P, NT*S]
    c1 = np.clip(r1.reshape(L, NH).astype(np.float64), 1e-7, None)
    c2 = np.clip(r2.reshape(L, NH).astype(np.float64), 1e-7, None)
    c3 = np.clip(r3.reshape(L, NH).astype(np.float64), 1e-7, None)
    idx = np.arange(S)
    rel = np.abs(idx[None, :] - idx[:, None]).astype(np.float64)  # [j, i]
    ekb = np.empty((L, NH, P, NT * S), NPBF16)
    for l in range(L):
        for h in range(NH):
            relp = np.where(rel > 0, rel, 1.0) ** c3[l, h]
            relp = np.where(rel > 0, relp, 0.0)
            m = np.exp(-c1[l, h] * np.log1p(c2[l, h] * relp))  # [j, i]
            # [j, i] -> [jt, p, i] -> [p, jt, i] -> [p, jt*i]
            ekb[l, h] = np.ascontiguousarray(
                m.reshape(NT, P, S).transpose(1, 0, 2).reshape(P, NT * S)
            ).astype(NPBF16)
    shared["ekb"] = ekb

    wq = Wqkv_w[:, :HID, :] / 8.0           # fold 1/sqrt(DH)
    wk = Wqkv_w[:, HID:2 * HID, :]
    bq = Wqkv_b[:, :HID] / 8.0
    bk = Wqkv_b[:, HID:2 * HID]
    wqk = np.concatenate([wq, wk], axis=1)  # [L, 1536, HID]
    wqkT = np.transpose(wqk, (0, 2, 1))     # [L, HID, 1536]
    wqk_p = np.ascontiguousarray(
        wqkT.reshape(L, KC, P, 2 * KC, P).transpose(0, 3, 2, 1, 4))
    bqk_p = np.concatenate([bq, bk], axis=1)  # [L, 1536]

    wv = Wqkv_w[:, 2 * HID:, :]             # [L, 768v, 768]
    bv = Wqkv_b[:, 2 * HID:]
    wva = np.zeros((L, HID, NH * 65), f32)
    bva_p = np.zeros((L, NH * 65), f32)
    for h in range(NH):
        wva[:, :, 65 * h:65 * h + 64] = np.transpose(
            wv[:, DH * h:DH * (h + 1), :], (0, 2, 1))
        bva_p[:, 65 * h:65 * h + 64] = bv[:, DH * h:DH * (h + 1)]
        bva_p[:, 65 * h + 64] = 1.0
    wva_p = np.ascontiguousarray(
        wva.reshape(L, KC, P, 2, HALF).transpose(0, 3, 2, 1, 4))

    woaT = np.transpose(attn_out_w, (0, 2, 1))  # [L, HID, HID]
    woa_p = np.ascontiguousarray(
        woaT.reshape(L, KC, P, KC, P).transpose(0, 3, 2, 1, 4))

    glu = np.empty((L, HID, NIC, 256), f32)
    gw = np.transpose(glu_w, (0, 2, 1))     # [L, HID, 6144]
    for gt in range(NIC):
        glu[:, :, gt, 0:128] = gw[:, :, gt * P:(gt + 1) * P]
        glu[:, :, gt, 128:256] = gw[:, :, INTER + gt * P:INTER + (gt + 1) * P]
    glu_p = np.ascontiguousarray(
        glu.reshape(L, KC, P, NIC, 256).transpose(0, 3, 2, 1, 4))
    wot_p = np.ascontiguousarray(np.transpose(wo_w, (0, 2, 1)))  # [L, INTER, HID]

    w16 = np.concatenate([
        wqk_p.reshape(L, -1), wva_p.reshape(L, -1), woa_p.reshape(L, -1),
        glu_p.reshape(L, -1), wot_p.reshape(L, -1),
        bqk_p.reshape(L, -1), bva_p.reshape(L, -1),
        attn_out_b.reshape(L, -1), wo_b.reshape(L, -1),
    ], axis=1).astype(NPBF16)
    shared["w16"] = np.ascontiguousarray(w16.reshape(-1))

    def pcol(v):  # [L, 768] -> [L, P, KC]
        return np.ascontiguousarray(v.reshape(L, KC, P).transpose(0, 2, 1)).astype(f32)

    lnp = np.stack([pcol(ln1_g), pcol(ln1_b), pcol(ln2_g), pcol(ln2_b)],
                   axis=1)  # [L, 4, P, KC]
    lnp = np.ascontiguousarray(lnp.transpose(2, 0, 1, 3)).reshape(P, 4 * L * KC)

    in_maps = []
    for b in range(B):
        m = dict(shared)
        hTb = np.ascontiguousarray(hidden_states[b].T).astype(f32)
        m["hT"] = hTb
        m["h16"] = hTb.astype(NPBF16)
        mask = attention_mask[b].astype(f32)          # [S]
        mbias = (1.0 - mask) * -10000.0
        c32 = np.zeros((P, NT + S + 2 + 4 * L * KC), f32)
        c32[:, 0:NT] = mbias.reshape(NT, P).T
        c32[:, NT:NT + S] = mask[None, :]
        c32[:, NT + S + 2:] = lnp
        m["c32"] = c32
        in_maps.append(m)
    return in_maps


def kernel(**inputs) -> np.ndarray:
    n_layers = int(inputs.pop("_n_layers", L))
    if n_layers not in _BUILT:
        _BUILT[n_layers] = _build(n_layers)
    nc = _BUILT[n_layers]
    in_maps = _prep_inputs(**inputs)
    res = run_bass_kernel_spmd(nc, in_maps, list(range(B))).results
    out = np.empty((B, S, HID), np.float32)
    for b in range(B):
        out[b] = res[b]["out"].T
    return out
